# revision 1
# baseline (speedup 1.0000x reference)
"""3-layer GCN + global mean/max pool + linear classifier on 8 Trainium2 NeuronCores.

Strategy (node-parallel, NOT edge-parallel):
  * Aggregate-first algebra: Agg(x @ W) == Agg(x) @ W, with the symmetric
    normalization folded into pre-scaled features  xt = dinv * x  so the
    per-edge multiply disappears:
        out[v] = dinv[v] * ( sum_{e: dst=v} xt[src_e] + xt[v] ) @ W + b
  * Shard dst nodes across 8 cores at graph-aligned boundaries (64 graphs /
    core).  Each core aggregates only its own ~12.5K dst rows (~400K edges),
    gathering source rows from a replicated feature table via indirect DMA
    (128 rows x 512B per "slot", batched ~1MB per DMA instruction).
  * Per-core dst nodes are sorted by in-degree so fixed-slot tiles have
    almost no padding.  All per-core variation (indices, scales, graph ids)
    is input data => one SPMD program for all cores.
  * After layers 1-2 each core's output shard is AllGather'd (in chunks, to
    overlap with compute) into the next layer's gather table.  Layer 3 output
    stays local: pooling only needs the core's own 64 graphs.
  * Sum-pool via PE matmul with a data-driven graph-indicator matrix; max-pool
    via a second small indirect gather pass over the local h3 scratch.

kernel(**inputs) takes the full unsharded inputs and returns the full
[512, 2] output.
"""

import os
import sys

import numpy as np

sys.path.insert(0, "/opt/trn_rl_repo")

N_CORES = 8
GPAIR = 8  # pair-slots per main gather DMA (gather buf = [128, GPAIR*256])
POOL_G = 16  # slots per pooling gather DMA
N_CHUNKS = 4  # allgather chunks per layer
TABLE_DT = os.environ.get("GCN_TABLE_DT", "float32")  # float32 | bfloat16

LAST_RESULTS = None  # BassKernelResults of the most recent run (for test.py)


# --------------------------------------------------------------------------
# host-side graph preprocessing
# --------------------------------------------------------------------------
def _host_prep(x, edge_index, batch, n_cores, n_graphs, n_chunks, shared_tables=False):
    """Compute all per-core index/scale arrays and the layout metadata.

    shared_tables: layer-2/3 gather tables are Shared-address-space DRAM
    written by a single mesh AllGather (needs n_cores > 4).  The zero row is
    then a pad row inside some core's stripe (pad rows compute to exact 0).
    """
    if shared_tables:
        n_chunks = 1
    x = np.asarray(x, dtype=np.float32)
    edge_index = np.asarray(edge_index)
    batch = np.asarray(batch).astype(np.int64)
    N, D = x.shape
    E = edge_index.shape[1]
    gpc = n_graphs // n_cores

    src = edge_index[0].astype(np.int64)
    dst = edge_index[1].astype(np.int64)
    indeg = np.bincount(dst, minlength=N).astype(np.int64)
    deg = (indeg + 1).astype(np.float64)  # + self loop
    dinv = (1.0 / np.sqrt(deg)).astype(np.float32)

    xt = x * dinv[:, None]
    xt = np.concatenate([xt, np.zeros((1, D), np.float32)], 0)  # zero row at N

    # graph boundaries (batch is sorted)
    gstart = np.searchsorted(batch, np.arange(n_graphs + 1)).astype(np.int64)
    B = gstart[np.arange(n_cores + 1) * gpc]  # core node boundaries

    # per-core degree-sorted permutation of owned nodes
    perms = []
    for i in range(n_cores):
        nodes = np.arange(B[i], B[i + 1])
        perms.append(nodes[np.argsort(-indeg[nodes], kind="stable")])
    n_own = np.array([len(p) for p in perms])
    n_tiles = int(np.ceil((n_own.max() + (1 if shared_tables else 0)) / 128))
    n_tiles += n_tiles % 2  # even, for pair supertiles
    S_rows = n_tiles * 128
    n_super = n_tiles // 2

    # localpos[v] = position of node v within its core's permuted layout
    localpos = np.zeros(N, dtype=np.int64)
    for p in perms:
        localpos[p] = np.arange(len(p))

    # slots per supertile: 1 (self) + max in-degree among its 256 nodes,
    # maxed across cores (static SPMD program).
    slots_u = np.ones(n_super, dtype=np.int64)
    for i in range(n_cores):
        dg = np.zeros(S_rows, dtype=np.int64)
        dg[: n_own[i]] = indeg[perms[i]]
        m = dg.reshape(n_super, 256).max(1)
        slots_u = np.maximum(slots_u, 1 + m)
    M = int(slots_u.max())
    col_of_super = np.concatenate([[0], np.cumsum(2 * slots_u)]).astype(np.int64)
    S_cols = int(col_of_super[-1])

    # edge lists sorted by dst, with per-dst rank
    eo = np.argsort(dst, kind="stable")
    ds, ss = dst[eo], src[eo]
    eptr = np.searchsorted(ds, np.arange(N + 1))
    erank = np.arange(E, dtype=np.int64) - eptr[ds]

    # chunk layout over supertiles
    chunk_supers = np.array_split(np.arange(n_super), n_chunks)
    chunk_u0 = [int(cs[0]) if len(cs) else 0 for cs in chunk_supers]
    chunk_rows = [len(cs) * 256 for cs in chunk_supers]
    chunk_off = np.concatenate([[0], np.cumsum(np.array(chunk_rows) * n_cores)])
    chunk_of_super = np.zeros(n_super, dtype=np.int64)
    for c, cs in enumerate(chunk_supers):
        chunk_of_super[cs] = c

    # table position of each node for layers 2/3
    chunk_rows_arr = np.array(chunk_rows, dtype=np.int64)
    chunk_base = np.array([chunk_u0[cc] * 256 for cc in range(n_chunks)], dtype=np.int64)

    def pos_of(core, r):
        c = chunk_of_super[np.asarray(r) // 256]
        return chunk_off[c] + core * chunk_rows_arr[c] + (np.asarray(r) - chunk_base[c])

    if shared_tables:
        # zero row = first pad row of the core with the most padding
        k0 = int(np.argmin(n_own))
        assert n_own[k0] < S_rows, "no pad rows available for the zero row"
        Z = int(pos_of(k0, n_own[k0]))
        T_rows = int(chunk_off[-1])
    else:
        Z = int(chunk_off[-1])  # extra zero row appended past all stripes
        T_rows = Z + 1

    pos23 = np.full(N + 1, Z, dtype=np.int64)
    for i in range(n_cores):
        r = np.arange(n_own[i])
        pos23[perms[i]] = pos_of(i, r)

    per_core = []
    for i in range(n_cores):
        # node-id grid [S_rows, M]; -1 = padding
        grid = np.full((S_rows, M), -1, dtype=np.int64)
        grid[: n_own[i], 0] = perms[i]
        m = (ds >= B[i]) & (ds < B[i + 1])
        grid[localpos[ds[m]], 1 + erank[m]] = ss[m]

        def grid_to_idx(posmap, padpos):
            cols = []
            for u in range(n_super):
                su = int(slots_u[u])
                sub = grid[u * 256 : (u + 1) * 256, :su]
                p = np.where(sub < 0, padpos, posmap[np.clip(sub, 0, None)])
                cols.append(
                    p.reshape(2, 128, su).transpose(1, 2, 0).reshape(128, 2 * su)
                )
            return np.ascontiguousarray(
                np.concatenate(cols, axis=1).astype(np.int32)
            )

        idx1 = grid_to_idx(np.arange(N + 1, dtype=np.int64), N)
        idx23 = grid_to_idx(pos23, Z)

        # per-tile scale columns (perm order, pads = 0)
        dv = np.zeros(S_rows, dtype=np.float32)
        dv[: n_own[i]] = dinv[perms[i]]
        d2c = np.ascontiguousarray((dv * dv).reshape(n_tiles, 128).T)
        d1c = np.ascontiguousarray(dv.reshape(n_tiles, 128).T)
        dvr = dv.reshape(1, S_rows)

        # graph id (local) of each perm row; pads = -1
        gl = np.full(S_rows, -1.0, dtype=np.float32)
        gl[: n_own[i]] = (batch[perms[i]] - i * gpc).astype(np.float32)
        gid = np.ascontiguousarray(gl.reshape(n_tiles, 128).T)

        per_core.append(
            dict(idx1=idx1, idx23=idx23, d2c=d2c, d1c=d1c, dvr=dvr, gid=gid)
        )

    # pooling: P_slots = max graph size (global); pool idx [gpc, P_slots]
    cnt_all = np.diff(gstart)
    P_slots = int(cnt_all.max())
    P_slots = -(-P_slots // POOL_G) * POOL_G  # round up to POOL_G
    for i in range(n_cores):
        pidx = np.full((gpc, P_slots), S_rows, dtype=np.int32)  # -inf row
        cnt = np.zeros(gpc, dtype=np.int64)
        for g in range(gpc):
            s, e = gstart[i * gpc + g], gstart[i * gpc + g + 1]
            cnt[g] = e - s
            pidx[g, : e - s] = localpos[np.arange(s, e)]
        per_core[i]["pidx"] = pidx
        ic = np.where(cnt > 0, 1.0 / np.maximum(cnt, 1), 0.0).astype(np.float32)
        per_core[i]["icnt"] = np.ascontiguousarray(
            np.broadcast_to(ic[None, :], (2, gpc))
        )

    return dict(
        N=N,
        D=D,
        gpc=gpc,
        n_cores=n_cores,
        n_tiles=n_tiles,
        n_super=n_super,
        S_rows=S_rows,
        S_cols=S_cols,
        slots_u=slots_u,
        col_of_super=col_of_super,
        chunk_supers=chunk_supers,
        chunk_u0=chunk_u0,
        chunk_rows=chunk_rows,
        chunk_off=chunk_off,
        Z=Z,
        T_rows=T_rows,
        P_slots=P_slots,
        xt=xt,
        per_core=per_core,
        n_chunks=n_chunks,
        shared_tables=shared_tables,
    )


# --------------------------------------------------------------------------
# device program
# --------------------------------------------------------------------------
def _build(prep, weights, table_dt_name="float32"):
    from concourse import bacc, bass, mybir, tile
    from concourse.masks import make_identity

    f32 = mybir.dt.float32
    i32 = mybir.dt.int32
    tdt = getattr(mybir.dt, table_dt_name)
    Alu = mybir.AluOpType
    Act = mybir.ActivationFunctionType

    D = prep["D"]
    gpc = prep["gpc"]
    n_cores = prep["n_cores"]
    n_tiles = prep["n_tiles"]
    n_super = prep["n_super"]
    S_rows = prep["S_rows"]
    S_cols = prep["S_cols"]
    slots_u = prep["slots_u"]
    col_of = prep["col_of_super"]
    chunk_supers = prep["chunk_supers"]
    chunk_rows = prep["chunk_rows"]
    chunk_off = prep["chunk_off"]
    Z = prep["Z"]
    T_rows = prep["T_rows"]
    P_slots = prep["P_slots"]
    n_chunks = prep["n_chunks"]
    N = prep["N"]

    dma_eng = os.environ.get("GCN_DMA", "gpsimd")

    global DMA_ENGINE
    def DMA_ENGINE(nc_):
        return getattr(nc_, dma_eng)

    nc = bacc.Bacc(
        "TRN2",
        target_bir_lowering=False,
        debug=False,
        enable_asserts=False,
        num_devices=n_cores,
    )

    # ---- I/O ----
    xt_d = nc.dram_tensor("xt", [N + 1, D], tdt, kind="ExternalInput")
    idx1_d = nc.dram_tensor("idx1", [128, S_cols], i32, kind="ExternalInput")
    idx23_d = nc.dram_tensor("idx23", [128, S_cols], i32, kind="ExternalInput")
    pidx_d = nc.dram_tensor("pidx", [gpc, P_slots], i32, kind="ExternalInput")
    d2c_d = nc.dram_tensor("d2c", [128, n_tiles], f32, kind="ExternalInput")
    d1c_d = nc.dram_tensor("d1c", [128, n_tiles], f32, kind="ExternalInput")
    dvr_d = nc.dram_tensor("dvr", [1, S_rows], f32, kind="ExternalInput")
    gid_d = nc.dram_tensor("gid", [128, n_tiles], f32, kind="ExternalInput")
    icnt_d = nc.dram_tensor("icnt", [2, gpc], f32, kind="ExternalInput")
    w_d = [
        nc.dram_tensor(f"w{l}", [D, D], f32, kind="ExternalInput") for l in (1, 2, 3)
    ]
    br_d = [
        nc.dram_tensor(f"b{l}r", [1, D], f32, kind="ExternalInput") for l in (1, 2, 3)
    ]
    wcm_d = nc.dram_tensor("wcm", [D, 2], f32, kind="ExternalInput")
    wcx_d = nc.dram_tensor("wcx", [D, 2], f32, kind="ExternalInput")
    bc2_d = nc.dram_tensor("bc2", [2, 1], f32, kind="ExternalInput")
    out_d = nc.dram_tensor("out", [2, gpc], f32, kind="ExternalOutput")
    debug = int(os.environ.get("GCN_DEBUG", "0") or "0")
    if debug:
        dbg_h3 = nc.dram_tensor("dbg_h3", [S_rows + 1, D], f32, kind="ExternalOutput")
        dbg_sum = nc.dram_tensor("dbg_sum", [128, gpc], f32, kind="ExternalOutput")
        dbg_max = nc.dram_tensor("dbg_max", [gpc, 128], f32, kind="ExternalOutput")
    if debug >= 2:
        dbg_t2 = nc.dram_tensor("dbg_t2", [T_rows, D], tdt, kind="ExternalOutput")
        dbg_t3 = nc.dram_tensor("dbg_t3", [T_rows, D], tdt, kind="ExternalOutput")

    with tile.TileContext(nc) as tc:
        with (
            tc.tile_pool(name="constp", bufs=1) as constp,
            tc.tile_pool(name="gbp", bufs=3) as gbp,
            tc.tile_pool(name="accp", bufs=3) as accp,
            tc.tile_pool(name="miscp", bufs=3) as miscp,
            tc.tile_pool(name="idxp", bufs=2) as idxp,
            tc.tile_pool(name="psp", bufs=2, space="PSUM") as psp,
            tc.tile_pool(name="pst_p", bufs=1, space="PSUM") as pst_p,
            tc.tile_pool(name="dramp", bufs=1, space="DRAM") as dramp,
        ):
            # ---- constants ----
            ident = constp.tile([128, 128], f32, name="ident")
            make_identity(nc, ident[:])
            w_sb = []
            for l in range(3):
                wt = constp.tile([D, D], f32, name=f"w{l}sb")
                DMA_ENGINE(nc).dma_start(out=wt[:], in_=w_d[l].ap())
                w_sb.append(wt)
            br_sb = []
            for l in range(3):
                bt = constp.tile([1, D], f32, name=f"b{l}sb")
                DMA_ENGINE(nc).dma_start(out=bt[:], in_=br_d[l].ap())
                br_sb.append(bt)
            dvr = constp.tile([1, S_rows], f32, name="dvr_sb")
            DMA_ENGINE(nc).dma_start(out=dvr[:], in_=dvr_d.ap())
            d2c = constp.tile([128, n_tiles], f32, name="d2c_sb")
            DMA_ENGINE(nc).dma_start(out=d2c[:], in_=d2c_d.ap())
            d1c = constp.tile([128, n_tiles], f32, name="d1c_sb")
            DMA_ENGINE(nc).dma_start(out=d1c[:], in_=d1c_d.ap())
            gid = constp.tile([128, n_tiles], f32, name="gid_sb")
            DMA_ENGINE(nc).dma_start(out=gid[:], in_=gid_d.ap())
            icnt = constp.tile([2, gpc], f32, name="icnt_sb")
            DMA_ENGINE(nc).dma_start(out=icnt[:], in_=icnt_d.ap())
            wcm = constp.tile([D, 2], f32, name="wcm_sb")
            DMA_ENGINE(nc).dma_start(out=wcm[:], in_=wcm_d.ap())
            wcx = constp.tile([D, 2], f32, name="wcx_sb")
            DMA_ENGINE(nc).dma_start(out=wcx[:], in_=wcx_d.ap())
            bc2 = constp.tile([2, 1], f32, name="bc2_sb")
            DMA_ENGINE(nc).dma_start(out=bc2[:], in_=bc2_d.ap())
            pidx = constp.tile([gpc, P_slots], i32, name="pidx_sb")
            DMA_ENGINE(nc).dma_start(out=pidx[:], in_=pidx_d.ap())
            ones1 = constp.tile([1, 128], f32, name="ones1")
            nc.vector.memset(ones1[:], 1.0)
            iotag = constp.tile([128, gpc], f32, name="iotag")
            nc.gpsimd.iota(
                iotag[:],
                pattern=[[1, gpc]],
                channel_multiplier=0,
                allow_small_or_imprecise_dtypes=True,
            )
            zrow = constp.tile([1, D], tdt, name="zrow")
            nc.vector.memset(zrow[:], 0.0)
            nrow = constp.tile([1, D], f32, name="nrow")
            nc.vector.memset(nrow[:], -3.0e38)
            sumT = constp.tile([128, gpc], f32, name="sumT")
            nc.vector.memset(sumT[:], 0.0)

            # ---- DRAM scratch ----
            shared_tables = prep["shared_tables"]
            tbl_space = "Shared" if shared_tables else "Local"
            table = {
                2: dramp.tile([T_rows, D], tdt, name="table2", addr_space=tbl_space),
                3: dramp.tile([T_rows, D], tdt, name="table3", addr_space=tbl_space),
            }
            h3s = dramp.tile([S_rows + 1, D], f32, name="h3s")
            bounce = {
                l: [
                    dramp.tile([chunk_rows[c], D], tdt, name=f"bnc{l}_{c}")
                    for c in range(n_chunks)
                ]
                for l in (2, 3)
            }
            if not shared_tables:
                DMA_ENGINE(nc).dma_start(out=table[2][Z : Z + 1, :], in_=zrow[:])
                DMA_ENGINE(nc).dma_start(out=table[3][Z : Z + 1, :], in_=zrow[:])
            DMA_ENGINE(nc).dma_start(out=h3s[S_rows : S_rows + 1, :], in_=nrow[:])

            # ---- three GCN layers ----
            for layer in (1, 2, 3):
                src_ap = xt_d.ap() if layer == 1 else table[layer]
                idx_dram = idx1_d if layer == 1 else idx23_d
                dcol = d2c if layer < 3 else d1c
                w = w_sb[layer - 1]
                br = br_sb[layer - 1]

                for c in range(n_chunks):
                    cs = chunk_supers[c]
                    if len(cs) == 0:
                        continue
                    u0, u1 = int(cs[0]), int(cs[-1]) + 1
                    cc0, cc1 = int(col_of[u0]), int(col_of[u1])
                    idxt = idxp.tile([128, cc1 - cc0], i32, tag="idxt")
                    DMA_ENGINE(nc).dma_start(out=idxt[:], in_=idx_dram.ap()[:, cc0:cc1])

                    for u in range(u0, u1):
                        su = int(slots_u[u])
                        base = int(col_of[u]) - cc0
                        acc = accp.tile([128, 256], f32, tag="acc")
                        # HW indirect DMA only honors ONE index per partition
                        # per instruction (multi-index APs stream consecutive
                        # rows instead) — issue one gather per slot-half.
                        done = 0
                        while done < su:
                            g = min(GPAIR, su - done)
                            gb = gbp.tile([128, GPAIR * 256], tdt, tag="gb")
                            for j in range(g):
                                for h2 in (0, 1):
                                    nc.gpsimd.indirect_dma_start(
                                        out=gb[
                                            :, j * 256 + h2 * 128 : j * 256 + (h2 + 1) * 128
                                        ],
                                        out_offset=None,
                                        in_=src_ap,
                                        in_offset=bass.IndirectOffsetOnAxis(
                                            ap=idxt[
                                                :,
                                                base + 2 * (done + j) + h2
                                                : base + 2 * (done + j) + h2 + 1,
                                            ],
                                            axis=0,
                                        ),
                                    )
                            for j in range(g):
                                sl = gb[:, j * 256 : (j + 1) * 256]
                                if done + j == 0:
                                    nc.vector.tensor_copy(out=acc[:], in_=sl)
                                else:
                                    nc.vector.tensor_tensor(
                                        out=acc[:], in0=acc[:], in1=sl, op=Alu.add
                                    )
                            done += g

                        for h in (0, 1):
                            t = 2 * u + h
                            diag = miscp.tile([128, 128], f32, tag="diag")
                            nc.vector.tensor_scalar_mul(
                                out=diag[:], in0=ident[:], scalar1=dcol[:, t : t + 1]
                            )
                            ps1 = psp.tile([128, 128], f32, tag="ps1")
                            nc.tensor.matmul(
                                out=ps1[:],
                                lhsT=acc[:, h * 128 : (h + 1) * 128],
                                rhs=diag[:],
                                start=True,
                                stop=True,
                            )
                            sT = miscp.tile([128, 128], f32, tag="sT")
                            nc.vector.tensor_copy(out=sT[:], in_=ps1[:])
                            ps2 = psp.tile([128, 128], f32, tag="ps2")
                            if layer < 3:
                                nc.tensor.matmul(
                                    out=ps2[:],
                                    lhsT=dvr[:, t * 128 : (t + 1) * 128],
                                    rhs=br[:],
                                    start=True,
                                    stop=False,
                                )
                                nc.tensor.matmul(
                                    out=ps2[:], lhsT=sT[:], rhs=w[:],
                                    start=False, stop=True,
                                )
                                tout = miscp.tile([128, 128], tdt, tag="tout")
                                nc.scalar.activation(
                                    out=tout[:], in_=ps2[:], func=Act.Relu
                                )
                                r0 = (t - 2 * u0) * 128
                                DMA_ENGINE(nc).dma_start(
                                    out=bounce[layer + 1][c][r0 : r0 + 128, :],
                                    in_=tout[:],
                                )
                            else:
                                nc.tensor.matmul(
                                    out=ps2[:], lhsT=ones1[:], rhs=br[:],
                                    start=True, stop=False,
                                )
                                nc.tensor.matmul(
                                    out=ps2[:], lhsT=sT[:], rhs=w[:],
                                    start=False, stop=True,
                                )
                                h3t = miscp.tile([128, 128], f32, tag="tout")
                                nc.vector.tensor_copy(out=h3t[:], in_=ps2[:])
                                DMA_ENGINE(nc).dma_start(
                                    out=h3s[t * 128 : (t + 1) * 128, :], in_=h3t[:]
                                )
                                stile = miscp.tile([128, gpc], f32, tag="stile")
                                nc.vector.tensor_tensor(
                                    out=stile[:],
                                    in0=gid[:, t : t + 1].to_broadcast([128, gpc]),
                                    in1=iotag[:],
                                    op=Alu.is_equal,
                                )
                                pst = pst_p.tile([128, gpc], f32, tag="pst")
                                nc.tensor.matmul(
                                    out=pst[:], lhsT=h3t[:], rhs=stile[:],
                                    start=True, stop=True,
                                )
                                nc.vector.tensor_tensor(
                                    out=sumT[:], in0=sumT[:], in1=pst[:], op=Alu.add
                                )

                    if layer < 3:
                        nc.gpsimd.collective_compute(
                            "AllGather",
                            Alu.bypass,
                            replica_groups=[list(range(n_cores))],
                            ins=[bounce[layer + 1][c][:].opt()],
                            outs=[
                                table[layer + 1][
                                    int(chunk_off[c]) : int(chunk_off[c])
                                    + n_cores * chunk_rows[c],
                                    :,
                                ].opt()
                            ],
                        )

            # ---- max pooling over local h3 ----
            maxacc = constp.tile([gpc, 128], f32, name="maxacc")
            done = 0
            while done < P_slots:
                g = min(POOL_G, P_slots - done)
                pgb = gbp.tile([gpc, POOL_G * 128], f32, tag="pgb")
                for j in range(g):
                    nc.gpsimd.indirect_dma_start(
                        out=pgb[:, j * 128 : (j + 1) * 128],
                        out_offset=None,
                        in_=h3s,
                        in_offset=bass.IndirectOffsetOnAxis(
                            ap=pidx[:, done + j : done + j + 1], axis=0
                        ),
                    )
                for j in range(g):
                    sl = pgb[:, j * 128 : (j + 1) * 128]
                    if done + j == 0:
                        nc.vector.tensor_copy(out=maxacc[:], in_=sl)
                    else:
                        nc.vector.tensor_tensor(
                            out=maxacc[:], in0=maxacc[:], in1=sl, op=Alu.max
                        )
                done += g
            psmT = pst_p.tile([128, gpc], f32, tag="psmT")
            nc.tensor.transpose(
                out=psmT[:], in_=maxacc[:], identity=ident[:gpc, :gpc]
            )
            maxT = miscp.tile([128, gpc], f32, tag="maxT")
            nc.vector.tensor_copy(out=maxT[:], in_=psmT[:])

            # ---- classifier ----
            psz1 = pst_p.tile([2, gpc], f32, tag="psz1")
            nc.tensor.matmul(out=psz1[:], lhsT=wcm[:], rhs=sumT[:], start=True, stop=True)
            psz2 = pst_p.tile([2, gpc], f32, tag="psz2")
            nc.tensor.matmul(out=psz2[:], lhsT=wcx[:], rhs=maxT[:], start=True, stop=True)
            zt = miscp.tile([2, gpc], f32, tag="zt")
            nc.vector.tensor_tensor(out=zt[:], in0=psz1[:], in1=icnt[:], op=Alu.mult)
            nc.vector.tensor_tensor(out=zt[:], in0=zt[:], in1=psz2[:], op=Alu.add)
            nc.vector.tensor_scalar_add(out=zt[:], in0=zt[:], scalar1=bc2[:, :1])
            DMA_ENGINE(nc).dma_start(out=out_d.ap(), in_=zt[:])

            if debug:
                # bounce whole tables / scratch through SBUF tiles to outputs
                def dump(dst_ap, src_ap, rows, width, dt_):
                    for r0 in range(0, rows, 128):
                        r1 = min(r0 + 128, rows)
                        buf = miscp.tile([128, width], dt_, tag="dbgbuf")
                        DMA_ENGINE(nc).dma_start(
                            out=buf[: r1 - r0, :], in_=src_ap[r0:r1, :]
                        )
                        DMA_ENGINE(nc).dma_start(
                            out=dst_ap[r0:r1, :], in_=buf[: r1 - r0, :]
                        )

                if debug >= 2:
                    dump(dbg_t2.ap(), table[2], T_rows, D, tdt)
                    dump(dbg_t3.ap(), table[3], T_rows, D, tdt)
                dump(dbg_h3.ap(), h3s, S_rows + 1, D, f32)
                dbuf = miscp.tile([128, gpc], f32, tag="dbgs")
                nc.vector.tensor_copy(out=dbuf[:], in_=sumT[:])
                DMA_ENGINE(nc).dma_start(out=dbg_sum.ap(), in_=dbuf[:])
                dbuf2 = miscp.tile([gpc, 128], f32, tag="dbgm")
                nc.vector.tensor_copy(out=dbuf2[:], in_=maxacc[:])
                DMA_ENGINE(nc).dma_start(out=dbg_max.ap(), in_=dbuf2[:])

    return nc


def _in_maps(prep, weights, table_dt_name):
    np_tdt = np.float32 if table_dt_name == "float32" else None
    xt = prep["xt"]
    if table_dt_name == "bfloat16":
        import ml_dtypes

        np_tdt = ml_dtypes.bfloat16
    xt = xt.astype(np_tdt)
    W1, b1, W2, b2, W3, b3, Wc, bc = weights
    maps = []
    for pc in prep["per_core"]:
        maps.append(
            {
                "xt": xt,
                "idx1": pc["idx1"],
                "idx23": pc["idx23"],
                "pidx": pc["pidx"],
                "d2c": pc["d2c"],
                "d1c": pc["d1c"],
                "dvr": pc["dvr"],
                "gid": pc["gid"],
                "icnt": pc["icnt"],
                "w1": np.asarray(W1, np.float32),
                "w2": np.asarray(W2, np.float32),
                "w3": np.asarray(W3, np.float32),
                "b1r": np.asarray(b1, np.float32).reshape(1, -1),
                "b2r": np.asarray(b2, np.float32).reshape(1, -1),
                "b3r": np.asarray(b3, np.float32).reshape(1, -1),
                "wcm": np.asarray(Wc, np.float32)[: prep["D"]],
                "wcx": np.asarray(Wc, np.float32)[prep["D"] :],
                "bc2": np.asarray(bc, np.float32).reshape(2, 1),
                "icnt": pc["icnt"],
            }
        )
    return maps


# --------------------------------------------------------------------------
# entry point
# --------------------------------------------------------------------------
def kernel(x, edge_index, batch, W1, b1, W2, b2, W3, b3, Wc, bc):
    global LAST_RESULTS
    from concourse import bass_utils

    n_graphs = 512
    shared = os.environ.get("GCN_SHARED", "0") == "1"
    prep = _host_prep(
        x, edge_index, batch, N_CORES, n_graphs, N_CHUNKS, shared_tables=shared
    )
    weights = (W1, b1, W2, b2, W3, b3, Wc, bc)
    nc = _build(prep, weights, TABLE_DT)
    nc.compile()
    maps = _in_maps(prep, weights, TABLE_DT)
    res = bass_utils.run_bass_kernel_spmd(
        nc,
        maps,
        core_ids=list(range(N_CORES)),
        trace=os.environ.get("GCN_TRACE") == "1",
    )
    LAST_RESULTS = res
    outs = [res.results[c]["out"] for c in range(N_CORES)]
    return np.concatenate([np.asarray(o, np.float32).T for o in outs], 0)



# revision 2
# speedup vs baseline: 1.1966x; 1.1966x over previous
"""3-layer GCN + global mean/max pool + linear on 8 Trainium2 NeuronCores.

v2: dma_gather-based (SWDGE bulk gather) node-parallel design.

  * Node-parallel: core i owns 64 graphs (~12.5K dst nodes).
  * Table layout: 4 windows of <=32768 rows (int16 gather index limit).
    Window w holds chunk-w rows of every core: [core0 seg | ... | core7 seg],
    each segment = (NT+1)*128 rows: NT compute tiles + one 128-row zero block
    (gather-pad target, spread across 8*128 distinct rows per window).
  * Node -> (chunk, rank) chosen by a greedy balance so each dst's in-edges
    split evenly across the 4 windows (minimizes slot-grid padding), then
    degree-sorted within each (core, chunk).
  * Per (tile, window): ONE dma_gather fetches all slot rows (slot-major
    order: idx i -> partition i%128 = dst lane, block i//128 = slot).
    fp16 table rows (256B). 4 SWDGE queues round-robin.
  * Reduction: fp16 tree-fold on DVE -> per-tile [128,128]; then the PSUM
    dance: transpose+scale via diag matmul, @W fp16, +d*b, relu -> fp16
    bounce rows; per-chunk AllGather into next layer's table.
  * Pooling: mean via PE indicator matmuls; max via dma_gather over local h3
    (lane = graph + 64*half) + max-fold + PE transpose + half-fold.
"""

import os
import sys

import numpy as np

sys.path.insert(0, "/opt/trn_rl_repo")

N_CORES = 8
NWIN = 4
WINBASE = 32768
LAST_RESULTS = None


# ------------------------------------------------------------------
# host-side graph prep
# ------------------------------------------------------------------
def _host_prep(x, edge_index, batch, n_cores, n_graphs, nwin, winbase, seed=0):
    x = np.asarray(x, dtype=np.float32)
    src = np.asarray(edge_index[0], dtype=np.int64)
    dst = np.asarray(edge_index[1], dtype=np.int64)
    batch = np.asarray(batch).astype(np.int64)
    N, D = x.shape
    E = len(src)
    gpc = n_graphs // n_cores

    indeg = np.bincount(dst, minlength=N).astype(np.int64)
    dinv = (1.0 / np.sqrt((indeg + 1).astype(np.float64))).astype(np.float32)
    xt = x * dinv[:, None]

    gstart = np.searchsorted(batch, np.arange(n_graphs + 1)).astype(np.int64)
    B = gstart[np.arange(n_cores + 1) * gpc]
    core_of = np.zeros(N, np.int8)
    for i in range(n_cores):
        core_of[B[i] : B[i + 1]] = i
    ncore = np.diff(B)

    NT = int(np.ceil(ncore.max() / (nwin * 128))) + 1  # compute tiles / chunk
    CAP = NT * 128
    SEG = (NT + 1) * 128  # + zero block
    assert n_cores * SEG <= winbase

    # ---- greedy window assignment (balance in-edges per dst) ----
    rng = np.random.default_rng(seed)
    so = np.argsort(src, kind="stable")
    ss, ds_ = src[so], dst[so]
    sptr = np.searchsorted(ss, np.arange(N + 1))
    cnt_wd = np.zeros((nwin, N), np.int32)
    capleft = np.full((n_cores, nwin), CAP, np.int64)
    win_of = np.zeros(N, np.int8)
    for p in range(3):
        order = rng.permutation(N)
        for v in order:
            a, b = sptr[v], sptr[v + 1]
            nbrs = ds_[a:b]
            ci = core_of[v]
            if p > 0:
                wo = win_of[v]
                capleft[ci, wo] += 1
                if len(nbrs):
                    cnt_wd[wo, nbrs] -= 1
            if len(nbrs):
                scores = cnt_wd[:, nbrs].sum(1).astype(np.int64)
            else:
                scores = np.zeros(nwin, np.int64)
            scores[capleft[ci] <= 0] = 1 << 40
            w = int(np.argmin(scores))
            win_of[v] = w
            capleft[ci, w] -= 1
            if len(nbrs):
                cnt_wd[w, nbrs] += 1

    # ---- positions: degree-desc sort within each (core, chunk) ----
    rank = np.zeros(N, np.int64)
    tile_of = np.zeros(N, np.int64)  # global tile id = chunk*NT + local tile
    lane_of = np.zeros(N, np.int64)
    n_cw = np.zeros((n_cores, nwin), np.int64)
    members = {}
    for i in range(n_cores):
        for w in range(nwin):
            nodes = np.where((core_of == i) & (win_of == w))[0]
            nodes = nodes[np.argsort(-indeg[nodes], kind="stable")]
            members[(i, w)] = nodes
            r = np.arange(len(nodes))
            rank[nodes] = r
            tile_of[nodes] = w * NT + r // 128
            lane_of[nodes] = r % 128
            n_cw[i, w] = len(nodes)

    # position inside window (gather index value)
    winpos = core_of.astype(np.int64) * SEG + rank
    NTILES = nwin * NT

    # ---- slot counts per (tile, window): global max over cores/lanes ----
    # c_tw[node, w] = in-edge count from window w (+1 self in own window)
    cnt_nw = cnt_wd.T.astype(np.int64).copy()  # [N, nwin]
    cnt_nw[np.arange(N), win_of.astype(np.int64)] += 1
    s_tw = np.zeros((NTILES, nwin), np.int64)
    for i in range(n_cores):
        for w in range(nwin):
            nodes = members[(i, w)]
            cw = np.zeros((NT * 128, nwin), np.int64)
            cw[: len(nodes)] = cnt_nw[nodes]
            m = cw.reshape(NT, 128, nwin).max(1)
            s_tw[w * NT : (w + 1) * NT] = np.maximum(
                s_tw[w * NT : (w + 1) * NT], m
            )
    s_tw = np.maximum(s_tw, 1)
    S_t = s_tw.sum(1)  # blocks per tile

    # zero-pad target rows inside each window
    zrows = {
        w: np.concatenate(
            [j * SEG + NT * 128 + np.arange(128) for j in range(n_cores)]
        )
        for w in range(nwin)
    }

    # ---- per-core index grids ----
    # edges grouped by (dstcore): rank of edge within (dst, window)
    eo = np.lexsort((ss, win_of[ss], ds_))  # sort by dst, then window, then src
    ds2, ss2 = ds_[eo], ss[eo]
    ws2 = win_of[ss2].astype(np.int64)
    # rank within (dst, window)
    key = ds2 * nwin + ws2
    kptr_idx = np.searchsorted(key, np.arange(N * nwin + 1))
    erank = np.arange(E, dtype=np.int64) - kptr_idx[key]
    # self-shift: edges in dst's own window shift +1 (self slot first)
    self_w = win_of[ds2].astype(np.int64)
    erank = erank + (ws2 == self_w)

    col_of_tw = np.zeros((NTILES, nwin), np.int64)  # block offset of (t,w) in tile
    c = np.zeros(NTILES, np.int64)
    for w in range(nwin):
        col_of_tw[:, w] = c
        c += s_tw[:, w]

    TOTBLK = int(S_t.sum())
    per_core = []
    for i in range(n_cores):
        grid = np.zeros((TOTBLK, 128), np.int32)  # slot-major rows of idx
        gfill = np.zeros((TOTBLK, 128), bool)
        tile_base = np.concatenate([[0], np.cumsum(S_t)])
        # pads default: round-robin zero rows per window
        for t in range(NTILES):
            for w in range(nwin):
                b0 = tile_base[t] + col_of_tw[t, w]
                sw = s_tw[t, w]
                z = zrows[w]
                fill = z[(np.arange(sw * 128) + (t * 131 + w * 37)) % len(z)]
                grid[b0 : b0 + sw] = fill.reshape(sw, 128)
        # self slots
        for w in range(nwin):
            nodes = members[(i, w)]
            t_loc = np.arange(len(nodes)) // 128
            b0 = tile_base[w * NT + t_loc] + col_of_tw[w * NT + t_loc, w]
            grid[b0, np.arange(len(nodes)) % 128] = winpos[nodes]
        # edges with dst in this core
        m = (ds2 >= B[i]) & (ds2 < B[i + 1])
        ed, es, ew, er = ds2[m], ss2[m], ws2[m], erank[m]
        t_g = tile_of[ed]
        b = tile_base[t_g] + col_of_tw[t_g, ew] + er
        grid[b, lane_of[ed]] = winpos[es]

        # wrap to [128, TOTBLK*8] int16 (16-partition wrap, replicated x8)
        flat = grid.reshape(TOTBLK * 128)
        wrapped = flat.reshape(-1, 16).T.astype(np.int16)  # [16, TOTBLK*8]
        gidx = np.tile(wrapped, (8, 1))

        # per-tile scale/graph columns in (chunk,tile) T order
        dv = np.zeros(NTILES * 128, np.float32)
        gl = np.full(NTILES * 128, -1.0, np.float32)
        for w in range(nwin):
            nodes = members[(i, w)]
            p = w * NT * 128 + np.arange(len(nodes))
            dv[p] = dinv[nodes]
            gl[p] = (batch[nodes] - i * gpc).astype(np.float32)
        d1c = np.ascontiguousarray(dv.reshape(NTILES, 128).T)
        d2c = np.ascontiguousarray((dv * dv).reshape(NTILES, 128).T)
        dvr = dv.reshape(1, NTILES * 128)
        gid = np.ascontiguousarray(gl.reshape(NTILES, 128).T)

        per_core.append(dict(gidx=gidx, d1c=d1c, d2c=d2c, dvr=dvr, gid=gid))

    # ---- xt table (fp16), node rows at (window, core, rank) positions ----
    TROWS = (nwin - 1) * winbase + n_cores * SEG
    xt_tab = np.zeros((TROWS, D), np.float16)
    tabpos = win_of.astype(np.int64) * winbase + winpos
    xt_tab[tabpos] = xt.astype(np.float16)

    # ---- pooling ----
    h3pos = tile_of * 128 + lane_of  # local h3 row of each node (per core)
    H3R = NTILES * 128 + 128  # + pad block
    gsz = np.diff(gstart)
    P2 = int(np.ceil(gsz.max() / 2))
    NPOOL = max(1, int(np.ceil(P2 * 128 / 4096)))
    psplit = np.array_split(np.arange(P2), NPOOL)
    for i in range(n_cores):
        pg = np.full((P2, 128), 0, np.int32)
        pfill = np.zeros((P2, 128), bool)
        padrows = NTILES * 128 + np.arange(128)
        pg[:] = padrows[(np.arange(P2 * 128) % 128)].reshape(P2, 128)
        for g in range(gpc):
            mem = np.arange(gstart[i * gpc + g], gstart[i * gpc + g + 1])
            j = np.arange(len(mem))
            pg[j // 2, g + 64 * (j % 2)] = h3pos[mem]
        flat = pg.reshape(P2 * 128)
        wrapped = flat.reshape(-1, 16).T.astype(np.int16)
        per_core[i]["pidx"] = np.tile(wrapped, (8, 1))
        cnt = gsz[i * gpc : (i + 1) * gpc].astype(np.float32)
        ic = np.where(cnt > 0, 1.0 / np.maximum(cnt, 1), 0.0).astype(np.float32)
        per_core[i]["icnt"] = np.ascontiguousarray(
            np.broadcast_to(ic[None, :], (2, gpc))
        )

    return dict(
        N=N, D=D, E=E, gpc=gpc, n_cores=n_cores, nwin=nwin, winbase=winbase,
        NT=NT, SEG=SEG, NTILES=NTILES, TOTBLK=TOTBLK, TROWS=TROWS,
        s_tw=s_tw, S_t=S_t, col_of_tw=col_of_tw,
        tile_base=np.concatenate([[0], np.cumsum(S_t)]),
        P2=P2, NPOOL=NPOOL, psplit=psplit, H3R=H3R,
        xt_tab=xt_tab, per_core=per_core,
    )


# ------------------------------------------------------------------
# device program
# ------------------------------------------------------------------
def _build(prep):
    from concourse import bacc, bass, mybir, tile, library_config
    from concourse.masks import make_identity

    f32 = mybir.dt.float32
    f16 = mybir.dt.float16
    i16 = mybir.dt.int16
    Alu = mybir.AluOpType
    Act = mybir.ActivationFunctionType

    D = prep["D"]
    gpc = prep["gpc"]
    n_cores = prep["n_cores"]
    nwin = prep["nwin"]
    winbase = prep["winbase"]
    NT = prep["NT"]
    SEG = prep["SEG"]
    NTILES = prep["NTILES"]
    TOTBLK = prep["TOTBLK"]
    TROWS = prep["TROWS"]
    s_tw = prep["s_tw"]
    S_t = prep["S_t"]
    col_of_tw = prep["col_of_tw"]
    tile_base = prep["tile_base"]
    P2 = prep["P2"]
    NPOOL = prep["NPOOL"]
    psplit = prep["psplit"]
    H3R = prep["H3R"]
    NQ = 4

    nc = bacc.Bacc(
        "TRN2",
        target_bir_lowering=False,
        debug=False,
        enable_asserts=False,
        num_devices=n_cores,
        num_swdge_queues=NQ,
    )

    xt_d = nc.dram_tensor("xt", [TROWS, D], f16, kind="ExternalInput")
    gidx_d = nc.dram_tensor("gidx", [128, TOTBLK * 8], i16, kind="ExternalInput")
    pidx_d = nc.dram_tensor("pidx", [128, P2 * 8], i16, kind="ExternalInput")
    d1c_d = nc.dram_tensor("d1c", [128, NTILES], f32, kind="ExternalInput")
    d2c_d = nc.dram_tensor("d2c", [128, NTILES], f32, kind="ExternalInput")
    dvr_d = nc.dram_tensor("dvr", [1, NTILES * 128], f16, kind="ExternalInput")
    gid_d = nc.dram_tensor("gid", [128, NTILES], f32, kind="ExternalInput")
    icnt_d = nc.dram_tensor("icnt", [2, gpc], f32, kind="ExternalInput")
    w_d = [nc.dram_tensor(f"w{l}", [D, D], f32, kind="ExternalInput") for l in (1, 2, 3)]
    br_d = [nc.dram_tensor(f"b{l}r", [1, D], f32, kind="ExternalInput") for l in (1, 2, 3)]
    wcm_d = nc.dram_tensor("wcm", [D, 2], f32, kind="ExternalInput")
    wcx_d = nc.dram_tensor("wcx", [D, 2], f32, kind="ExternalInput")
    bc2_d = nc.dram_tensor("bc2", [2, 1], f32, kind="ExternalInput")
    out_d = nc.dram_tensor("out", [2, gpc], f32, kind="ExternalOutput")
    debug = int(os.environ.get("GCN2_DEBUG", "0") or "0")
    if debug:
        dbg_h3 = nc.dram_tensor("dbg_h3", [H3R, D], f16, kind="ExternalOutput")
        dbg_t2 = nc.dram_tensor("dbg_t2", [TROWS, D], f16, kind="ExternalOutput")

    with tile.TileContext(nc) as tc:
        with (
            tc.tile_pool(name="constp", bufs=1) as constp,
            tc.tile_pool(name="gbp", bufs=6) as gbp,
            tc.tile_pool(name="idxp", bufs=4) as idxp,
            tc.tile_pool(name="accp", bufs=4) as accp,
            tc.tile_pool(name="miscp", bufs=4) as miscp,
            tc.tile_pool(name="pgbp", bufs=1) as pgbp,
            tc.tile_pool(name="psp", bufs=2, space="PSUM") as psp,
            tc.tile_pool(name="pstp", bufs=1, space="PSUM") as pstp,
            tc.tile_pool(name="dramp", bufs=1, space="DRAM") as dramp,
        ):
            nc.gpsimd.load_library(library_config.mlp)

            # ---- constants ----
            ident = constp.tile([128, 128], f32, name="ident")
            make_identity(nc, ident[:])
            w_sb, br_sb = [], []
            for l in range(3):
                wt32 = miscp.tile([D, D], f32, tag="wld")
                nc.sync.dma_start(out=wt32[:], in_=w_d[l].ap())
                wt = constp.tile([D, D], f16, name=f"w{l}sb")
                nc.vector.tensor_copy(out=wt[:], in_=wt32[:])
                w_sb.append(wt)
                bt32 = miscp.tile([1, D], f32, tag="bld")
                nc.sync.dma_start(out=bt32[:], in_=br_d[l].ap())
                bt = constp.tile([1, D], f16, name=f"b{l}sb")
                nc.vector.tensor_copy(out=bt[:], in_=bt32[:])
                br_sb.append(bt)
            d1c32 = constp.tile([128, NTILES], f32, name="d1c32")
            nc.sync.dma_start(out=d1c32[:], in_=d1c_d.ap())
            d2c32 = constp.tile([128, NTILES], f32, name="d2c32")
            nc.sync.dma_start(out=d2c32[:], in_=d2c_d.ap())
            dvr = constp.tile([1, NTILES * 128], f16, name="dvr")
            nc.sync.dma_start(out=dvr[:], in_=dvr_d.ap())
            gid = constp.tile([128, NTILES], f32, name="gid")
            nc.sync.dma_start(out=gid[:], in_=gid_d.ap())
            icnt = constp.tile([2, gpc], f32, name="icnt")
            nc.sync.dma_start(out=icnt[:], in_=icnt_d.ap())
            wcm = constp.tile([D, 2], f32, name="wcm")
            nc.sync.dma_start(out=wcm[:], in_=wcm_d.ap())
            wcx = constp.tile([D, 2], f32, name="wcx")
            nc.sync.dma_start(out=wcx[:], in_=wcx_d.ap())
            bc2 = constp.tile([2, 1], f32, name="bc2")
            nc.sync.dma_start(out=bc2[:], in_=bc2_d.ap())
            ones1 = constp.tile([1, 128], f16, name="ones1")
            nc.vector.memset(ones1[:], 1.0)
            iotag = constp.tile([128, gpc], f32, name="iotag")
            nc.gpsimd.iota(
                iotag[:], pattern=[[1, gpc]], channel_multiplier=0,
                allow_small_or_imprecise_dtypes=True,
            )
            zero16 = constp.tile([128, D], f16, name="zero16")
            nc.vector.memset(zero16[:], 0.0)
            maxneg = constp.tile([128, D], f16, name="maxneg")
            nc.vector.memset(maxneg[:], -60000.0)
            sumT = constp.tile([128, gpc], f32, name="sumT")
            nc.vector.memset(sumT[:], 0.0)

            # ---- DRAM scratch ----
            table = {
                2: dramp.tile([TROWS, D], f16, name="table2"),
                3: dramp.tile([TROWS, D], f16, name="table3"),
            }
            bounce = {
                l: [dramp.tile([SEG, D], f16, name=f"bnc{l}_{c}") for c in range(nwin)]
                for l in (2, 3)
            }
            h3s = dramp.tile([H3R, D], f16, name="h3s")
            nc.sync.dma_start(out=h3s[NTILES * 128 : NTILES * 128 + 128, :], in_=maxneg[:])

            qctr = [0]

            def next_q():
                q = qctr[0] % NQ
                qctr[0] += 1
                return q

            # ---- GCN layers ----
            for layer in (1, 2, 3):
                if layer == 1:
                    win_src = [
                        xt_d.ap()[w * winbase : w * winbase + n_cores * SEG, :]
                        for w in range(nwin)
                    ]
                else:
                    win_src = [
                        table[layer][w * winbase : w * winbase + n_cores * SEG, :]
                        for w in range(nwin)
                    ]
                w_mat = w_sb[layer - 1]
                br = br_sb[layer - 1]
                dcol = d2c32 if layer < 3 else d1c32

                for T in range(NTILES):
                    chunk = T // NT
                    t_in = T % NT
                    S = int(S_t[T])
                    b0 = int(tile_base[T])
                    gb = gbp.tile([128, S * 128], f16, tag="gb")
                    idxt = idxp.tile([128, S * 8], i16, tag="idxt")
                    nc.sync.dma_start(
                        out=idxt[:], in_=gidx_d.ap()[:, b0 * 8 : (b0 + S) * 8]
                    )
                    for w in range(nwin):
                        sw = int(s_tw[T, w])
                        cw = int(col_of_tw[T, w])
                        nc.gpsimd.dma_gather(
                            gb[:, cw * 128 : (cw + sw) * 128].rearrange(
                                "p (s e) -> p s e", e=128
                            ),
                            win_src[w],
                            idxt[:, cw * 8 : (cw + sw) * 8],
                            sw * 128,
                            sw * 128,
                            D,
                            queue_num=next_q(),
                            single_packet=False,
                        )
                    # reduce S slot-blocks -> acc f32 [128,128] in one op
                    acc = accp.tile([128, 128], f32, tag="acc")
                    nc.vector.tensor_reduce(
                        out=acc[:],
                        in_=gb[:].rearrange("p (s e) -> p e s", e=128),
                        axis=mybir.AxisListType.X,
                        op=Alu.add,
                    )
                    # PSUM dance: ps1 = acc^T * diag(d)
                    diag = miscp.tile([128, 128], f32, tag="diag")
                    nc.vector.tensor_scalar_mul(
                        out=diag[:], in0=ident[:], scalar1=dcol[:, T : T + 1]
                    )
                    ps1 = psp.tile([128, 128], f32, tag="ps1")
                    nc.tensor.matmul(
                        out=ps1[:], lhsT=acc[:], rhs=diag[:], start=True, stop=True
                    )
                    sT = miscp.tile([128, 128], f16, tag="sT")
                    nc.scalar.activation(out=sT[:], in_=ps1[:], func=Act.Copy)
                    ps2 = psp.tile([128, 128], f32, tag="ps2")
                    if layer < 3:
                        nc.tensor.matmul(
                            out=ps2[:],
                            lhsT=dvr[:, T * 128 : (T + 1) * 128],
                            rhs=br[:],
                            start=True,
                            stop=False,
                        )
                    else:
                        nc.tensor.matmul(
                            out=ps2[:], lhsT=ones1[:], rhs=br[:], start=True, stop=False
                        )
                    nc.tensor.matmul(
                        out=ps2[:], lhsT=sT[:], rhs=w_mat[:], start=False, stop=True
                    )
                    if layer < 3:
                        tout = miscp.tile([128, 128], f16, tag="tout")
                        nc.scalar.activation(out=tout[:], in_=ps2[:], func=Act.Relu)
                        nc.sync.dma_start(
                            out=bounce[layer + 1][chunk][
                                t_in * 128 : (t_in + 1) * 128, :
                            ],
                            in_=tout[:],
                        )
                    else:
                        h3t = miscp.tile([128, 128], f16, tag="tout")
                        nc.scalar.activation(out=h3t[:], in_=ps2[:], func=Act.Copy)
                        nc.sync.dma_start(
                            out=h3s[T * 128 : (T + 1) * 128, :], in_=h3t[:]
                        )
                        stile = miscp.tile([128, gpc], f16, tag="stile")
                        nc.vector.tensor_tensor(
                            out=stile[:],
                            in0=gid[:, T : T + 1].to_broadcast([128, gpc]),
                            in1=iotag[:],
                            op=Alu.is_equal,
                        )
                        pst = pstp.tile([128, gpc], f32, tag="pst")
                        nc.tensor.matmul(
                            out=pst[:], lhsT=h3t[:], rhs=stile[:], start=True, stop=True
                        )
                        nc.vector.tensor_tensor(
                            out=sumT[:], in0=sumT[:], in1=pst[:], op=Alu.add
                        )

                    if layer < 3 and t_in == NT - 1:
                        c = chunk
                        nc.sync.dma_start(
                            out=bounce[layer + 1][c][NT * 128 : (NT + 1) * 128, :],
                            in_=zero16[:],
                        )
                        nc.gpsimd.collective_compute(
                            "AllGather",
                            Alu.bypass,
                            replica_groups=[list(range(n_cores))],
                            ins=[bounce[layer + 1][c][:, :].opt()],
                            outs=[
                                table[layer + 1][
                                    c * winbase : c * winbase + n_cores * SEG, :
                                ].opt()
                            ],
                        )

            # ---- max pooling ----
            pgb = pgbp.tile([128, P2 * 128], f16, tag="pgb")
            pidxt = idxp.tile([128, P2 * 8], i16, tag="pidxt")
            nc.sync.dma_start(out=pidxt[:], in_=pidx_d.ap())
            for blocks in psplit:
                a, b = int(blocks[0]), int(blocks[-1]) + 1
                nc.gpsimd.dma_gather(
                    pgb[:, a * 128 : b * 128].rearrange("p (s e) -> p s e", e=128),
                    h3s[:, :],
                    pidxt[:, a * 8 : b * 8],
                    (b - a) * 128,
                    (b - a) * 128,
                    D,
                    queue_num=next_q(),
                    single_packet=False,
                )
            maxacc = miscp.tile([128, 128], f32, tag="maxacc")
            nc.vector.tensor_reduce(
                out=maxacc[:],
                in_=pgb[:].rearrange("p (s e) -> p e s", e=128),
                axis=mybir.AxisListType.X,
                op=Alu.max,
            )
            psmT = pstp.tile([128, 128], f32, tag="psmT")
            nc.tensor.transpose(out=psmT[:], in_=maxacc[:], identity=ident[:])
            mfull = miscp.tile([128, 128], f32, tag="mfull")
            nc.vector.tensor_copy(out=mfull[:], in_=psmT[:])
            maxT = miscp.tile([128, gpc], f32, tag="maxT")
            nc.vector.tensor_tensor(
                out=maxT[:], in0=mfull[:, :gpc], in1=mfull[:, gpc : 2 * gpc], op=Alu.max
            )

            # ---- classifier ----
            psz1 = pstp.tile([2, gpc], f32, tag="psz1")
            nc.tensor.matmul(out=psz1[:], lhsT=wcm[:], rhs=sumT[:], start=True, stop=True)
            psz2 = pstp.tile([2, gpc], f32, tag="psz2")
            nc.tensor.matmul(out=psz2[:], lhsT=wcx[:], rhs=maxT[:], start=True, stop=True)
            zt = miscp.tile([2, gpc], f32, tag="zt")
            nc.vector.tensor_tensor(out=zt[:], in0=psz1[:], in1=icnt[:], op=Alu.mult)
            nc.vector.tensor_tensor(out=zt[:], in0=zt[:], in1=psz2[:], op=Alu.add)
            nc.vector.tensor_scalar_add(out=zt[:], in0=zt[:], scalar1=bc2[:, :1])
            nc.sync.dma_start(out=out_d.ap(), in_=zt[:])

            if debug:
                for r0 in range(0, H3R, 128):
                    buf = miscp.tile([128, D], f16, tag="dbgb")
                    nc.sync.dma_start(out=buf[:], in_=h3s[r0 : r0 + 128, :])
                    nc.sync.dma_start(out=dbg_h3.ap()[r0 : r0 + 128, :], in_=buf[:])
                for r0 in range(0, TROWS, 128):
                    buf = miscp.tile([128, D], f16, tag="dbgb")
                    nc.sync.dma_start(out=buf[:], in_=table[2][r0 : r0 + 128, :])
                    nc.sync.dma_start(out=dbg_t2.ap()[r0 : r0 + 128, :], in_=buf[:])

    return nc


def _in_maps(prep, weights):
    W1, b1, W2, b2, W3, b3, Wc, bc = weights
    D = prep["D"]
    maps = []
    for pc in prep["per_core"]:
        maps.append(
            {
                "xt": prep["xt_tab"],
                "gidx": pc["gidx"],
                "pidx": pc["pidx"],
                "d1c": pc["d1c"],
                "d2c": pc["d2c"],
                "dvr": pc["dvr"].astype(np.float16),
                "gid": pc["gid"],
                "icnt": pc["icnt"],
                "w1": np.asarray(W1, np.float32),
                "w2": np.asarray(W2, np.float32),
                "w3": np.asarray(W3, np.float32),
                "b1r": np.asarray(b1, np.float32).reshape(1, -1),
                "b2r": np.asarray(b2, np.float32).reshape(1, -1),
                "b3r": np.asarray(b3, np.float32).reshape(1, -1),
                "wcm": np.asarray(Wc, np.float32)[:D],
                "wcx": np.asarray(Wc, np.float32)[D:],
                "bc2": np.asarray(bc, np.float32).reshape(2, 1),
            }
        )
    return maps


def _run(x, edge_index, batch, W1, b1, W2, b2, W3, b3, Wc, bc,
         n_cores=N_CORES, n_graphs=512):
    global LAST_RESULTS
    from concourse import bass_utils

    prep = _host_prep(x, edge_index, batch, n_cores, n_graphs, NWIN, WINBASE)
    nc = _build(prep)
    nc.compile()
    maps = _in_maps(prep, (W1, b1, W2, b2, W3, b3, Wc, bc))
    res = bass_utils.run_bass_kernel_spmd(
        nc,
        maps,
        core_ids=list(range(n_cores)),
        trace=os.environ.get("GCN_TRACE") == "1",
    )
    LAST_RESULTS = res
    outs = [res.results[c]["out"] for c in range(n_cores)]
    out = np.concatenate([np.asarray(o, np.float32).T for o in outs], 0)
    return out, prep, res


def kernel(x, edge_index, batch, W1, b1, W2, b2, W3, b3, Wc, bc):
    out, _, _ = _run(x, edge_index, batch, W1, b1, W2, b2, W3, b3, Wc, bc)
    return out
